# revision 1
# baseline (speedup 1.0000x reference)
"""Trainium2 Bass kernel for causal GQA attention (B=2, T=2048, E=2048, H=16, D=128, KVH=8).

Sharding: 8 cores = 2 (batch) x 4 (head groups). Each core computes 4 query heads
(column-parallel wq) + their 2 KV heads, full causal attention for those heads, and
a partial output projection (row-parallel wo). Host sums the 4 partials per batch.

Layout strategy: everything head-transposed ([D, T] with D on partitions) so that
no on-chip transposes are needed anywhere:
  - qT/kT = wq/wk.T @ x.T directly from PE (lhsT = weight slice, rhs = x.T)
  - scores S^T[k, q] = kT.T @ qT (lhsT = kT block, rhs = qT chunk)
  - attn_outT[d, q] = v_nat.T @ expS^T (lhsT = v natural [t, d], rhs = exp block)
  - out[t, e] = attn_outT.T @ wo (lhsT = attn_outT, rhs = wo rows)

Projection + output GEMMs run in compensated fp8 (hi+lo e4m3 splits, 3 terms
a_hi@b_hi + a_lo@b_hi + a_hi@b_lo) with DoubleRow pairing over the contraction:
0.75 cycles per bf16-pair-cycle => 25% PE savings at better-than-bf16 accuracy.
Weights are pre-scaled by powers of 2 (wq/wk/wo x64, wv x32) so their sigma~0.02
values escape e4m3's denormal range; the scales fold exactly into the exp scale
(/4096), the denominator ones-vector (32) and a final host divide (/64).
Attention (S, exp, AV, denominators) stays bf16.

Schedule: the K projection runs eo-pair-outer across all 8 (kv-head, chunk) units
(8 PSUM banks) so PE consumes x chunks as the input DMA delivers them instead of
idling through the ~25us input transfer. V and Q run chunk-major eo-inner
(consecutive same-bank accumulation). Attention runs chunk-descending (3,2,1,0)
with each chunk's output projection interleaved one chunk later, so the final
un-overlapped tail is the smallest chunk's wo group, whose PSUM->SBUF copies
alternate ACT/DVE (ACT is idle by then).

RoPE pairs are de-interleaved by permuting wq/wk columns on the host (scores are
invariant since q and k use the same permutation), so rope becomes a half-swap.
Softmax has no max-subtraction (logits are O(5)); causal masking is a 0/1
multiply on the exp'd diagonal blocks, full blocks above the diagonal skipped.
Denominators via ones-vector matmuls accumulated in PSUM alongside AV.

`reps`: wraps the whole body (including input DMA) in an on-device For_i loop -
used only for latency-slope timing in test.py; the graded path uses reps=1.
"""

import numpy as np
import ml_dtypes

BF16 = ml_dtypes.bfloat16
F8 = ml_dtypes.float8_e4m3
F16 = np.float16

B, T, E = 2, 2048, 2048
H, D = 16, 128
KVH = 8
THETA = 10000.0
P = 128
EP = E // (2 * P)    # 8 contraction eo-pairs (DoubleRow processes 2x128 per pass)
CH = 512             # q-chunk width
NTQ = T // CH        # 4 q chunks
NTB = T // P         # 16 t blocks
NH = H // 4          # 4 q heads per core
NKV = 2              # kv heads per core
SCALE = float(D) ** -0.5
WS = 64.0            # wq/wk/wo host prescale (pow2, exact)
VS = 32.0            # wv host prescale

_NC_CACHE = {}
_PHASE_LIMIT = "full"   # "proj" | "attn" | "full" — phase-cost probing only
_PROJ_BF16 = False      # probe: bf16 (non-DR) projections with same schedule
_NO_WAVE = False        # probe: K chunk-major (idle DMA head, dense PE bursts)
_ATTN_BF16 = False      # probe: attention+wo exactly as the known-good baseline


def _build_nc(reps=1):
    import concourse.mybir as mybir
    import concourse.tile as tile
    from concourse import bacc

    nc = bacc.Bacc(None, target_bir_lowering=False)
    dt = mybir.dt
    f32, bf16, f16, f8 = dt.float32, dt.bfloat16, dt.float16, dt.float8e4
    Exp = mybir.ActivationFunctionType.Exp
    Copy = mybir.ActivationFunctionType.Copy
    DR = mybir.MatmulPerfMode.DoubleRow

    xh_d = nc.dram_tensor("xh", [E, T], f8, kind="ExternalInput")
    xl_d = nc.dram_tensor("xl", [E, T], f8, kind="ExternalInput")
    wqh_d = nc.dram_tensor("wqh", [E, NH * D], f8, kind="ExternalInput")
    wql_d = nc.dram_tensor("wql", [E, NH * D], f8, kind="ExternalInput")
    wkh_d = nc.dram_tensor("wkh", [E, NKV * D], f8, kind="ExternalInput")
    wkl_d = nc.dram_tensor("wkl", [E, NKV * D], f8, kind="ExternalInput")
    wvh_d = nc.dram_tensor("wvh", [E, NKV * D], f8, kind="ExternalInput")
    wvl_d = nc.dram_tensor("wvl", [E, NKV * D], f8, kind="ExternalInput")
    if _ATTN_BF16:
        wo16_d = nc.dram_tensor("wo16", [NH * D, E], bf16, kind="ExternalInput")
        wo16_r = wo16_d.rearrange("(h p) e -> p h e", p=P)
    else:
        woh_d = nc.dram_tensor("woh", [NH * D, E], f8, kind="ExternalInput")
        wol_d = nc.dram_tensor("wol", [NH * D, E], f8, kind="ExternalInput")
    cos_d = nc.dram_tensor("cosd", [P, T], f16, kind="ExternalInput")
    sin_d = nc.dram_tensor("sind", [P, T], f16, kind="ExternalInput")
    mk_d = nc.dram_tensor("mkd", [4, P, CH], f8, kind="ExternalInput")
    o_d = nc.dram_tensor("od", [T, E], bf16, kind="ExternalOutput")

    xh_r = xh_d.rearrange("(ep two p) t -> p ep two t", p=P, two=2)
    xl_r = xl_d.rearrange("(ep two p) t -> p ep two t", p=P, two=2)
    wqh_r = wqh_d.rearrange("(ep two p) m -> p ep two m", p=P, two=2)
    wql_r = wql_d.rearrange("(ep two p) m -> p ep two m", p=P, two=2)
    wkh_r = wkh_d.rearrange("(ep two p) m -> p ep two m", p=P, two=2)
    wkl_r = wkl_d.rearrange("(ep two p) m -> p ep two m", p=P, two=2)
    wvh_r = wvh_d.rearrange("(ep two p) m -> p ep two m", p=P, two=2)
    wvl_r = wvl_d.rearrange("(ep two p) m -> p ep two m", p=P, two=2)
    if not _ATTN_BF16:
        woh_r = woh_d.rearrange("(pr two p) e -> p pr two e", p=P, two=2)
        wol_r = wol_d.rearrange("(pr two p) e -> p pr two e", p=P, two=2)
    mk_r = mk_d.rearrange("f p c -> p f c")
    o_r = o_d.rearrange("(tb p) e -> p tb e", p=P)

    with tile.TileContext(nc) as tc:
        with (
            tc.tile_pool(name="singles", bufs=1) as sg,
            tc.tile_pool(name="ropet", bufs=2) as rp,
            tc.tile_pool(name="expp", bufs=3) as ep_,
            tc.tile_pool(name="e8pp", bufs=2) as e8_,
            tc.tile_pool(name="normp", bufs=1) as np_,
            tc.tile_pool(name="scrp", bufs=1) as sc_,
            tc.tile_pool(name="outst", bufs=2) as op_,
        ):

            def emit_body():
                wkh_sb = sg.tile([P, EP, 2, NKV * D], f8, name="wkh_sb", tag="wkh_sb")
                wkl_sb = sg.tile([P, EP, 2, NKV * D], f8, name="wkl_sb", tag="wkl_sb")
                xh_sb = sg.tile([P, EP, 2, T], f8, name="xh_sb", tag="xh_sb")
                xl_sb = sg.tile([P, EP, 2, T], f8, name="xl_sb", tag="xl_sb")
                wvh_sb = sg.tile([P, EP, 2, NKV * D], f8, name="wvh_sb", tag="wvh_sb")
                wvl_sb = sg.tile([P, EP, 2, NKV * D], f8, name="wvl_sb", tag="wvl_sb")
                wqh_sb = sg.tile([P, EP, 2, NH * D], f8, name="wqh_sb", tag="wqh_sb")
                wql_sb = sg.tile([P, EP, 2, NH * D], f8, name="wql_sb", tag="wql_sb")
                cos_sb = sg.tile([P, T], f16, name="cos_sb", tag="cos_sb")
                sin_sb = sg.tile([P, T], f16, name="sin_sb", tag="sin_sb")
                if _ATTN_BF16:
                    wo16_sb = sg.tile([P, NH, E], bf16, name="wo16_sb", tag="wo16_sb")
                else:
                    woh_sb = sg.tile([P, 2, 2, E], f8, name="woh_sb", tag="woh_sb")
                    wol_sb = sg.tile([P, 2, 2, E], f8, name="wol_sb", tag="wol_sb")
                mk_sb = sg.tile([P, 4, CH], f8, name="mk_sb", tag="mk_sb")

                # All input DMAs share the DMA engines' aggregate bandwidth, so
                # one queue in strict consumption order: wk first (the K wave
                # needs it), then x in 1MB chunks (the wave's pacing stream,
                # hi/lo interleaved per eo-pair), rope tables mid-stream (needed
                # right at wave end), then wv/wq/mk/wo (consumed later).
                nc.sync.dma_start(wkh_sb[:], wkh_r[:])
                nc.sync.dma_start(wkl_sb[:], wkl_r[:])
                # first eo-pair at fine grain so the wave starts ASAP
                nc.sync.dma_start(xh_sb[:, 0:1], xh_r[:, 0:1])
                nc.sync.dma_start(xl_sb[:, 0:1], xl_r[:, 0:1])
                nc.sync.dma_start(xh_sb[:, 1:2], xh_r[:, 1:2])
                nc.sync.dma_start(xl_sb[:, 1:2], xl_r[:, 1:2])
                for e2 in range(1, EP // 2):
                    sl2 = slice(2 * e2, 2 * e2 + 2)
                    nc.sync.dma_start(xh_sb[:, sl2], xh_r[:, sl2])
                    nc.sync.dma_start(xl_sb[:, sl2], xl_r[:, sl2])
                    if e2 == 2:
                        nc.sync.dma_start(cos_sb[:], cos_d[:])
                        nc.sync.dma_start(sin_sb[:], sin_d[:])
                nc.sync.dma_start(wvh_sb[:], wvh_r[:])
                nc.sync.dma_start(wvl_sb[:], wvl_r[:])
                nc.sync.dma_start(wqh_sb[:], wqh_r[:])
                nc.sync.dma_start(wql_sb[:], wql_r[:])
                nc.sync.dma_start(mk_sb[:], mk_r[:])
                if _ATTN_BF16:
                    nc.sync.dma_start(wo16_sb[:], wo16_r[:])
                else:
                    nc.sync.dma_start(woh_sb[:], woh_r[:])
                    nc.sync.dma_start(wol_sb[:], wol_r[:])

                ones_sb = sg.tile([P, 1], bf16, name="ones_sb", tag="ones_sb")
                nc.vector.memset(ones_sb[:], VS)
                ones8_sb = sg.tile([P, 1], f8, name="ones8_sb", tag="ones8_sb")
                nc.vector.memset(ones8_sb[:], VS)

                kT_sb = [sg.tile([P, T], bf16, name=f"kT{g}", tag=f"kT{g}") for g in range(NKV)]
                qT_sb = [sg.tile([P, T], bf16, name=f"qT{h}", tag=f"qT{h}") for h in range(NH)]
                v_sb = sg.tile([P, NTB, NKV * D], bf16, name="v_sb", tag="v_sb")
                # fp8 hi/lo v, packed by t-block pairs for DoubleRow AV
                v8h_sb = sg.tile([P, NTB // 2, 2, NKV * D], f8, name="v8h_sb", tag="v8h_sb")
                v8l_sb = sg.tile([P, NTB // 2, 2, NKV * D], f8, name="v8l_sb", tag="v8l_sb")
                # attention outputs, fp8 hi/lo, packed [pair, two, T] for DoubleRow wo
                if not _ATTN_BF16:
                    aoh_sb = sg.tile([P, 2, 2, T], f8, name="aoh_sb", tag="aoh_sb")
                    aol_sb = sg.tile([P, 2, 2, T], f8, name="aol_sb", tag="aol_sb")

                def rope_chunk(dest, sl, ps):
                    # dest[:, sl] = ps * cos + swap_halves(ps) * sin (sin rows 0:64 pre-negated)
                    t1 = rp.tile([P, CH], f32, name="ropet1", tag="ropet1")
                    nc.vector.tensor_mul(t1[:], ps[:], cos_sb[:, sl])
                    t2 = rp.tile([P, CH], f32, name="ropet2", tag="ropet2")
                    nc.vector.tensor_mul(t2[0:64, :], ps[64:128, :], sin_sb[0:64, sl])
                    nc.vector.tensor_mul(t2[64:128, :], ps[0:64, :], sin_sb[64:128, sl])
                    nc.vector.tensor_add(dest[:, sl], t1[:], t2[:])

                def mm3(ps, wh, wl, rh, rl, start, stop):
                    # compensated fp8 pair: wh@rh + wl@rh + wh@rl (DoubleRow)
                    if _PROJ_BF16:
                        # probe: same math, DR off (2 single-tile matmuls/term)
                        for i, (a, b) in enumerate(((wh, rh), (wl, rh), (wh, rl))):
                            for two in range(2):
                                nc.tensor.matmul(
                                    ps, a[:, two, :], b[:, two, :],
                                    start=(start and i == 0 and two == 0),
                                    stop=(stop and i == 2 and two == 1))
                        return
                    nc.tensor.matmul(ps, wh, rh, start=start, stop=False, perf_mode=DR)
                    nc.tensor.matmul(ps, wl, rh, start=False, stop=False, perf_mode=DR)
                    nc.tensor.matmul(ps, wh, rl, start=False, stop=stop, perf_mode=DR)

                # ---- projections share one 8-slot PSUM ring: the K wave holds
                # all 8 slots eo-pair-outer (DMA-paced), then V/Q tiles cycle
                # through slots as each K rope frees one (no pool barrier).
                with tc.tile_pool(name="pjw", bufs=8, space="PSUM") as pjw:
                    if _NO_WAVE:
                        for g in range(NKV):
                            csl = slice(D * g, D * (g + 1))
                            for tci in range(NTQ):
                                xsl = slice(CH * tci, CH * (tci + 1))
                                psk1 = pjw.tile([P, CH], f32, name="psk", tag="pjw")
                                for ep in range(EP):
                                    mm3(psk1[:],
                                        wkh_sb[:, ep, :, csl], wkl_sb[:, ep, :, csl],
                                        xh_sb[:, ep, :, xsl], xl_sb[:, ep, :, xsl],
                                        start=(ep == 0), stop=(ep == EP - 1))
                                rope_chunk(kT_sb[g], xsl, psk1)
                    else:
                        psk = [[pjw.tile([P, CH], f32, name=f"psk{g}_{t}", tag="pjw")
                                for t in range(NTQ)] for g in range(NKV)]
                        for ep in range(EP):
                            for g in range(NKV):
                                csl = slice(D * g, D * (g + 1))
                                for tci in range(NTQ):
                                    xsl = slice(CH * tci, CH * (tci + 1))
                                    mm3(psk[g][tci][:],
                                        wkh_sb[:, ep, :, csl], wkl_sb[:, ep, :, csl],
                                        xh_sb[:, ep, :, xsl], xl_sb[:, ep, :, xsl],
                                        start=(ep == 0), stop=(ep == EP - 1))
                        for g in range(NKV):
                            for tci in range(NTQ):
                                rope_chunk(kT_sb[g], slice(CH * tci, CH * (tci + 1)),
                                           psk[g][tci])

                    # ---- V projection: chunk-major, eo-pair-inner (same-bank runs)
                    for u in range(8):
                        psv = pjw.tile([P, CH], f32, name="psv", tag="pjw")
                        for k2 in range(2):
                            tb = 2 * u + k2
                            tsl = slice(P * tb, P * (tb + 1))
                            for ep in range(EP):
                                mm3(psv[:, 256 * k2:256 * (k2 + 1)],
                                    xh_sb[:, ep, :, tsl], xl_sb[:, ep, :, tsl],
                                    wvh_sb[:, ep], wvl_sb[:, ep],
                                    start=(ep == 0), stop=(ep == EP - 1))
                        # ACT: idle during projections, DVE is busy with rope.
                        # psv holds t-blocks (2u, 2u+1) side by side = exactly
                        # v_sb[:, 2u:2u+2] and the pair-packed v8 slots for u.
                        nc.scalar.activation(v_sb[:, 2 * u:2 * u + 2, :], psv[:], Copy)
                        nc.scalar.activation(v8h_sb[:, u], psv[:], Copy)
                        nc.vector.tensor_sub(v8l_sb[:, u], psv[:], v8h_sb[:, u])

                with (
                    tc.tile_pool(name="pj", bufs=2, space="PSUM") as pj,
                    tc.tile_pool(name="ps_s", bufs=3, space="PSUM") as ps_s,
                    tc.tile_pool(name="ps_o", bufs=2, space="PSUM") as ps_o,
                    tc.tile_pool(name="ps_m", bufs=1, space="PSUM") as ps_m,
                ):
                    # ---- Q projection: chunk-major per head, on the attention
                    # pj ring (the pool barrier above only waits on V's copies;
                    # attention then overlaps Q's DVE rope tail with no stall)
                    for h in range(NH):
                        csl = slice(D * h, D * (h + 1))
                        for tci in range(NTQ):
                            xsl = slice(CH * tci, CH * (tci + 1))
                            psq = pj.tile([P, CH], f32, name="psq", tag="pj")
                            for ep in range(EP):
                                mm3(psq[:],
                                    wqh_sb[:, ep, :, csl], wql_sb[:, ep, :, csl],
                                    xh_sb[:, ep, :, xsl], xl_sb[:, ep, :, xsl],
                                    start=(ep == 0), stop=(ep == EP - 1))
                            rope_chunk(qT_sb[h], xsl, psq)

                    if _PHASE_LIMIT == "proj":
                        for h in range(NH):
                            nc.sync.dma_start(o_r[:, 4 * h, :], qT_sb[h][:])
                        for g in range(NKV):
                            nc.sync.dma_start(o_r[:, 8 + g, :], kT_sb[g][:])
                        return

                    # one PSUM bank holds both s_row accumulators: consecutive
                    # units alternate base partition 0/32 so the next unit's
                    # denominator matmuls never wait on the previous copy-out
                    # (PE executes in order, so that wait would stall everything)
                    if not _ATTN_BF16:
                        srow_bank = ps_m.tile([64, CH], f32, name="srow_bank", tag="srow_bank")
                    unit_idx = [0]

                    def attn(h, tci):
                        g = h // 2
                        sl = slice(CH * tci, CH * (tci + 1))
                        ntk = 4 * tci + 4
                        o_ps = ps_o.tile([P, CH], f32, name="o_ps", tag="o_ps")
                        if _ATTN_BF16:
                            s_row = ps_m.tile([1, CH], f32, name="s_row", tag="s_row")
                        else:
                            p0 = 32 * (unit_idx[0] % 2)
                            unit_idx[0] += 1
                            s_row = srow_bank[p0:p0 + 1, :]
                        escale = SCALE / (WS * WS)
                        # full blocks (j < 4*tci) in fp8 pairs: exp writes e4m3
                        # halves of a pair tile, then DoubleRow AV (v8 hi+lo)
                        # and one DoubleRow denominator matmul per pair
                        nfull = 4 * tci if not _ATTN_BF16 else 0
                        for pi in range(nfull // 2):
                            e8p = e8_.tile([P, 2, CH], f8, name="e8p", tag="e8p")
                            for half in range(2):
                                j = 2 * pi + half
                                s_ps = ps_s.tile([P, CH], f32, name="s_ps", tag="s_ps")
                                nc.tensor.matmul(
                                    s_ps[:], kT_sb[g][:, P * j:P * (j + 1)], qT_sb[h][:, sl],
                                    start=True, stop=True,
                                )
                                nc.scalar.activation(e8p[:, half], s_ps[:], Exp, scale=escale)
                            vsl = slice(D * g, D * (g + 1))
                            nc.tensor.matmul(
                                o_ps[:], v8h_sb[:, pi, :, vsl], e8p[:],
                                start=(pi == 0), stop=False, perf_mode=DR,
                            )
                            nc.tensor.matmul(
                                o_ps[:], v8l_sb[:, pi, :, vsl], e8p[:],
                                start=False, stop=False, perf_mode=DR,
                            )
                            for half in range(2):
                                nc.tensor.matmul(
                                    s_row[:], ones8_sb[:], e8p[:, half],
                                    start=(pi == 0 and half == 0), stop=False,
                                )
                        for j in range(nfull, ntk):
                            # diagonal blocks, bf16: only columns >= 128*di live
                            di = j - 4 * tci
                            c0 = P * di if di > 0 else 0
                            qsl = slice(CH * tci + c0, CH * (tci + 1))
                            s_ps = ps_s.tile([P, CH], f32, name="s_ps", tag="s_ps")
                            nc.tensor.matmul(
                                s_ps[:, c0:], kT_sb[g][:, P * j:P * (j + 1)], qT_sb[h][:, qsl],
                                start=True, stop=True,
                            )
                            e_t = ep_.tile([P, CH], bf16, name="e_t", tag="e_t")
                            nc.scalar.activation(e_t[:, c0:], s_ps[:, c0:], Exp, scale=escale)
                            if di >= 0:
                                nc.vector.tensor_mul(e_t[:, c0:], e_t[:, c0:], mk_sb[:, di, c0:])
                            nc.tensor.matmul(
                                o_ps[:, c0:], v_sb[:, j, D * g:D * (g + 1)], e_t[:, c0:],
                                start=(j == 0), stop=(j == ntk - 1),
                            )
                            nc.tensor.matmul(
                                s_row[:, c0:], ones_sb[:], e_t[:, c0:],
                                start=(j == 0), stop=(j == ntk - 1),
                            )
                        # normalize chain spread across Pool/DVE/ACT (DVE is the
                        # scarce engine during attention)
                        srow_sb = np_.tile([1, CH], f32, name="srow_sb", tag="srow_sb")
                        nc.vector.tensor_copy(out=srow_sb[:], in_=s_row[:])
                        rec = np_.tile([1, CH], f32, name="rec", tag="rec")
                        nc.vector.reciprocal(rec[:], srow_sb[:])
                        bc = np_.tile([P, CH], f32, name="bc", tag="bc")
                        nc.gpsimd.partition_broadcast(bc[:], rec[:])
                        if _ATTN_BF16:
                            nc.vector.tensor_mul(qT_sb[h][:, sl], o_ps[:], bc[:])
                            return
                        t_scr = sc_.tile([P, CH], f32, name="t_scr", tag="t_scr")
                        nc.vector.tensor_mul(t_scr[:], o_ps[:], bc[:])
                        pr, two = divmod(h, 2)
                        nc.scalar.activation(aoh_sb[:, pr, two, sl], t_scr[:], Copy)
                        nc.vector.tensor_sub(aol_sb[:, pr, two, sl], t_scr[:],
                                             aoh_sb[:, pr, two, sl])

                    def wo_group_bf16(tci, last=False):
                        for tb in range(4 * tci, 4 * tci + 4):
                            ost = op_.tile([P, E], bf16, name="ost", tag="ost")
                            for n in range(4):
                                nsl = slice(CH * n, CH * (n + 1))
                                wop = pj.tile([P, CH], f32, name="wop", tag="pj")
                                for h in range(NH):
                                    nc.tensor.matmul(
                                        wop[:],
                                        qT_sb[h][:, P * tb:P * (tb + 1)],
                                        wo16_sb[:, h, nsl],
                                        start=(h == 0), stop=(h == NH - 1),
                                    )
                                if last and n % 2 == 0:
                                    nc.scalar.activation(ost[:, nsl], wop[:], Copy)
                                else:
                                    nc.vector.tensor_copy(out=ost[:, nsl], in_=wop[:])
                            nc.sync.dma_start(o_r[:, tb, :], ost[:])

                    # wo: 6 DoubleRow matmuls per (tb, n): 2 head-pairs x 3 comp terms
                    def wo_group_fp8(tci, last=False):
                        for tb in range(4 * tci, 4 * tci + 4):
                            tsl = slice(P * tb, P * (tb + 1))
                            ost = op_.tile([P, E], bf16, name="ost", tag="ost")
                            for n in range(4):
                                nsl = slice(CH * n, CH * (n + 1))
                                wop = pj.tile([P, CH], f32, name="wop", tag="pj")
                                for pr in range(2):
                                    mm3(wop[:],
                                        aoh_sb[:, pr, :, tsl], aol_sb[:, pr, :, tsl],
                                        woh_sb[:, pr, :, nsl], wol_sb[:, pr, :, nsl],
                                        start=(pr == 0), stop=(pr == 1))
                                # PSUM->SBUF copies: GPSIMD can't read PSUM, so
                                # DVE with an ACT share (ACT-heavy on the last
                                # group, where exps are done)
                                on_act = (n % 2 == 0) if last else (n == 3)
                                if on_act:
                                    nc.scalar.activation(ost[:, nsl], wop[:], Copy)
                                else:
                                    nc.vector.tensor_copy(out=ost[:, nsl], in_=wop[:])
                                if last and n == 1:
                                    nc.sync.dma_start(o_r[:, tb, 0:2 * CH], ost[:, 0:2 * CH])
                            if last:
                                nc.sync.dma_start(o_r[:, tb, 2 * CH:], ost[:, 2 * CH:])
                            else:
                                nc.sync.dma_start(o_r[:, tb, :], ost[:])

                    wo_group = wo_group_bf16 if _ATTN_BF16 else wo_group_fp8

                    if _PHASE_LIMIT == "attn":
                        for tci in range(NTQ):
                            for h in range(NH):
                                attn(h, tci)
                        if _ATTN_BF16:
                            for h in range(NH):
                                nc.sync.dma_start(o_r[:, 4 * h, :], qT_sb[h][:])
                            return
                        aohb = aoh_sb[:].bitcast(bf16)
                        aolb = aol_sb[:].bitcast(bf16)
                        for pr in range(2):
                            for two in range(2):
                                nc.sync.dma_start(o_r[:, 2 * pr + two, 0:T // 2],
                                                  aohb[:, pr, two, :])
                                nc.sync.dma_start(o_r[:, 4 + 2 * pr + two, 0:T // 2],
                                                  aolb[:, pr, two, :])
                        return

                    # chunk-ascending attention; each chunk's wo interleaved one
                    # chunk later so the softmax normalize chain has drained.
                    for tci in range(NTQ):
                        for h in range(NH):
                            attn(h, tci)
                            if h == 0 and tci > 0:
                                wo_group(tci - 1)
                    wo_group(NTQ - 1, last=True)

            if reps > 1:
                with tc.For_i(0, reps, 1):
                    emit_body()
            else:
                emit_body()

    nc.finalize()
    return nc


def get_nc(reps=1):
    if reps not in _NC_CACHE:
        _NC_CACHE[reps] = _build_nc(reps)
    return _NC_CACHE[reps]


def _split8(a):
    hi = a.astype(F8)
    lo = (a - hi.astype(np.float32)).astype(F8)
    return hi, lo


def make_host_inputs(x, wq, wk, wv, wo):
    """Returns per-core in_maps (list of 8 dicts)."""
    perm = np.concatenate([np.arange(0, D, 2), np.arange(1, D, 2)])
    wq4 = np.asarray(wq, np.float32).reshape(E, H, D)[:, :, perm] * np.float32(WS)
    wk4 = np.asarray(wk, np.float32).reshape(E, KVH, D)[:, :, perm] * np.float32(WS)
    wv4 = np.asarray(wv, np.float32).reshape(E, KVH, D) * np.float32(VS)
    wo4 = np.asarray(wo, np.float32).reshape(H, D, E) * np.float32(WS)
    xT = np.ascontiguousarray(np.transpose(np.asarray(x, np.float32), (0, 2, 1)))
    xTh, xTl = _split8(xT)

    # rope tables (fp16; sin rows 0:64 pre-negated for the half-swap form)
    invf = 1.0 / (np.float32(THETA) ** (np.arange(0, D, 2, dtype=np.float32) / np.float32(D)))
    ang = np.arange(T, dtype=np.float32)[None, :] * invf[:, None]     # [64, T]
    cosv = np.cos(ang).astype(np.float32)
    sinv = np.sin(ang).astype(np.float32)
    cos_h = np.concatenate([cosv, cosv], 0).astype(F16)
    sin_h = np.concatenate([-sinv, sinv], 0).astype(F16)

    ii = np.arange(P)[:, None]
    jj = np.arange(CH)[None, :]
    mk_h = np.stack([(jj >= ii + P * di) for di in range(4)]).astype(F8)

    in_maps = []
    for c in range(8):
        b, hg = divmod(c, 4)
        qs = slice(4 * hg, 4 * hg + 4)
        ks = slice(2 * hg, 2 * hg + 2)
        wqc = np.ascontiguousarray(wq4[:, qs].reshape(E, NH * D))
        wkc = np.ascontiguousarray(wk4[:, ks].reshape(E, NKV * D))
        wvc = np.ascontiguousarray(wv4[:, ks].reshape(E, NKV * D))
        woc = np.ascontiguousarray(wo4[qs].reshape(NH * D, E))
        wqh, wql = _split8(wqc)
        wkh, wkl = _split8(wkc)
        wvh, wvl = _split8(wvc)
        woh, wol = _split8(woc)
        in_maps.append({
            "xh": xTh[b], "xl": xTl[b],
            "wqh": wqh, "wql": wql,
            "wkh": wkh, "wkl": wkl,
            "wvh": wvh, "wvl": wvl,
            "woh": woh, "wol": wol,
            "wo16": woc.astype(BF16),
            "cosd": cos_h,
            "sind": sin_h,
            "mkd": mk_h,
        })
    return in_maps


def gather_results(per_core_od):
    """Sum per-core partials and undo the wo host prescale."""
    out = np.zeros((B, T, E), np.float32)
    for c in range(8):
        out[c // 4] += per_core_od[c].astype(np.float32)
    out *= np.float32(1.0 / WS)
    return out


def kernel(x, mask, wq, wk, wv, wo, **extra):
    from concourse.bass_utils import run_bass_kernel_spmd

    nc = get_nc()
    in_maps = make_host_inputs(x, wq, wk, wv, wo)
    res = run_bass_kernel_spmd(nc, in_maps, core_ids=list(range(8)))
    return gather_results([res.results[c]["od"] for c in range(8)])

